# revision 1
# baseline (speedup 1.0000x reference)
"""Trainium2 Bass kernel for nn_NestedFormula.

Tree: DEPTH=4, V=4. Level sizes n4=1, n3=5, n2=25, n1=125, n0=125.
  f1[n] = sum_v lam1[n,v] * x_v^pow1[n,v] + lam0[n]
  fd[n] = sum_v lamd[n,v] * x_v^powd[n,v] * f_{d-1}[5n+v] + f_{d-1}[5n+4]
  out   = f4[0]                          (per batch element)

Strategy (pure data parallel over batch, 8 cores x 16384):
  - x^p = exp(p * ln x): one packed Ln + per-partition-scaled Exp calls on the
    scalar engine (the bottleneck: ~500+100+20+4 exps per batch elem).
  - (node,var) pairs live on partitions; batch on the free dim in 4 chunks
    of 4096. Levels 3/4 pack the chunk index into partitions too.
  - All weighted reductions are matmuls with host-precomputed block-diagonal
    G matrices (float32r for full PE rate). Gather patterns, last-subformula
    passthroughs (exp(0)=1 rows with weight-1 columns) and the lam0 bias
    (ones-row in the last L1 tile) are all folded into the G columns.
  - ln(x) is computed once packed (128,512), bounced to DRAM, and
    broadcast-read into replicated layouts with step-0 DMA access patterns.
"""
import numpy as np

import concourse.bacc as bacc
import concourse.mybir as mybir
from concourse.tile import TileContext

DEPTH = 4
V = 4
B = 131072
M_CORES = 8
BS = B // M_CORES          # 16384 per core
CHUNK = 4096
NCH = BS // CHUNK          # 4
HALF = 2048                # psum granularity
MMN = 512                  # matmul free dim (one PSUM bank)

F32 = mybir.dt.float32
F32R = mybir.dt.float32r

N1, N2, N3, N4 = 125, 25, 5, 1
J1 = 4 * N1                # 500 level-1 (node,var) pairs
NT1 = 4                    # level-1 j-tiles of 128


def _sigma1(m):
    # psum1 row m -> level-1 node index
    if m < 100:
        return 5 * (m // 4) + (m % 4)
    return 5 * (m - 100) + 4


def _tau2(m):
    # psum2 row m -> level-2 node index
    if m < 20:
        return 5 * (m // 4) + (m % 4)
    return 5 * (m - 20) + 4


def build_constants(lam0, lam1, pow1, lam2, pow2, lam3, pow3, lam4, pow4):
    c = {}
    # ---- level 1: 4 j-tiles of K=128, M=128 (125 used cols) ----
    sc1 = np.zeros((128, NT1), np.float32)
    g1 = np.zeros((NT1, 128, 128), np.float32)
    for n in range(N1):
        for v in range(V):
            j = 4 * n + v
            t, r = divmod(j, 128)
            sc1[r, t] = pow1[n, v]
    for m in range(125):
        n = _sigma1(m)
        for v in range(V):
            j = 4 * n + v
            t, r = divmod(j, 128)
            g1[t, r, m] = lam1[n, v]
        g1[3, 116, m] = lam0[n]          # ones-row (sc1[116,3]=0 -> exp=1)
    c["sc1"] = sc1
    c["g1"] = np.ascontiguousarray(g1.transpose(1, 0, 2).reshape(128, NT1 * 128))

    # ---- level 2: K=128 (100 exp rows + 25 passthrough), M=32 (25 used) ----
    sc2 = np.zeros((128, 1), np.float32)
    g2 = np.zeros((128, 32), np.float32)
    for n in range(N2):
        for v in range(V):
            sc2[4 * n + v, 0] = pow2[n, v]
    for m in range(25):
        n2t = _tau2(m)
        for v in range(V):
            g2[4 * n2t + v, m] = lam2[n2t, v]
        g2[100 + n2t, m] = 1.0           # + f1[5*n2t+4] passthrough
    c["sc2"] = sc2
    c["g2"] = g2

    # ---- level 3 (chunk-packed): rows 32c+m2, cols 5c+u ----
    sc3 = np.zeros((128, 1), np.float32)
    g3 = np.zeros((128, 32), np.float32)
    for cc in range(NCH):
        for m2 in range(25):
            r = 32 * cc + m2
            if m2 < 20:
                n3, v3 = divmod(m2, 4)
                sc3[r, 0] = pow3[n3, v3]
                g3[r, 5 * cc + n3] = lam3[n3, v3]
            else:
                g3[r, 5 * cc + (m2 - 20)] = 1.0   # + f2[5*n3+4]
    c["sc3"] = sc3
    c["g3"] = g3

    # ---- level 4 (chunk-packed): rows 5c+u, cols c ----
    sc4 = np.zeros((32, 1), np.float32)
    g4 = np.zeros((32, NCH), np.float32)
    for cc in range(NCH):
        for u in range(4):
            sc4[5 * cc + u, 0] = pow4[0, u]
            g4[5 * cc + u, cc] = lam4[0, u]
        g4[5 * cc + 4, cc] = 1.0                  # + f3[4]
    c["sc4"] = sc4
    c["g4"] = g4
    return c


def build_bass():
    nc = bacc.Bacc()
    xt = nc.dram_tensor("xt", (V, BS), F32, kind="ExternalInput")
    sc1 = nc.dram_tensor("sc1", (128, NT1), F32, kind="ExternalInput")
    g1 = nc.dram_tensor("g1", (128, NT1 * 128), F32R, kind="ExternalInput")
    sc2 = nc.dram_tensor("sc2", (128, 1), F32, kind="ExternalInput")
    g2 = nc.dram_tensor("g2", (128, 32), F32R, kind="ExternalInput")
    sc3 = nc.dram_tensor("sc3", (128, 1), F32, kind="ExternalInput")
    g3 = nc.dram_tensor("g3", (128, 32), F32R, kind="ExternalInput")
    sc4 = nc.dram_tensor("sc4", (32, 1), F32, kind="ExternalInput")
    g4 = nc.dram_tensor("g4", (32, NCH), F32R, kind="ExternalInput")
    y = nc.dram_tensor("y", (BS,), F32, kind="ExternalOutput")

    EXP = mybir.ActivationFunctionType.Exp
    LN = mybir.ActivationFunctionType.Ln

    with TileContext(nc) as tc:
        with tc.tile_pool(name="const", bufs=1) as cpool, \
             tc.tile_pool(name="dram", bufs=1, space="DRAM") as dpool, \
             tc.tile_pool(name="big", bufs=1) as bpool, \
             tc.tile_pool(name="psum", bufs=2, space="PSUM") as ppool:
            # one shared pool of 16KB/partition slots; phase-B tiles reuse
            # phase-A slots via tags (disjoint lifetimes)
            lpool = e1pool = e2pool = spool = bpool

            # ---------- constants into SBUF (G's cast to f32r) ----------
            sct1 = cpool.tile([128, NT1], F32)
            nc.sync.dma_start(out=sct1[:], in_=sc1[:, :])
            sct2 = cpool.tile([128, 1], F32)
            nc.sync.dma_start(out=sct2[:], in_=sc2[:, :])
            sct3 = cpool.tile([128, 1], F32)
            nc.sync.dma_start(out=sct3[:], in_=sc3[:, :])
            sct4 = cpool.tile([32, 1], F32)
            nc.sync.dma_start(out=sct4[:], in_=sc4[:, :])

            def load_g(dram_t, shape, tag):
                r = cpool.tile(list(shape), F32R, tag=tag)
                nc.sync.dma_start(out=r[:], in_=dram_t[:, :])
                return r

            g1t = load_g(g1, (128, NT1 * 128), "g1t")
            g2t = load_g(g2, (128, 32), "g2t")
            g3t = load_g(g3, (128, 32), "g3t")
            g4t = load_g(g4, (32, NCH), "g4t")

            # ---------- ln(x) packed, bounce to DRAM (per-chunk pipelined) ----
            # xc rows 4c''+v = x[v, 512c'':512(c''+1)]; chunk c = rows 32c..
            xc = cpool.tile([128, 512], F32, tag="xc")
            lc = cpool.tile([128, 512], F32, tag="lc")
            # per-chunk DRAM scratch tiles for ln(x) -> exact DMA deps
            lds = []
            for cc in range(NCH):
                r0 = 32 * cc
                xt_view = xt[:, cc * CHUNK:(cc + 1) * CHUNK] \
                    .rearrange("v (c i) -> c v i", i=512)
                nc.sync.dma_start(out=xc[r0:r0 + 32, :], in_=xt_view)
                nc.scalar.activation(lc[r0:r0 + 32, :], xc[r0:r0 + 32, :], LN)
                ldc = dpool.tile([V, CHUNK], F32, tag=f"ld{cc}")
                nc.sync.dma_start(
                    out=ldc[:, :].rearrange("v (c i) -> c v i", i=512),
                    in_=lc[r0:r0 + 32, :])
                lds.append(ldc)


            # f2all rows 32c+m2 = f2-stage values of chunk c
            f2all = spool.tile([128, CHUNK], F32, tag="f2all", bufs=1)

            # ---------- phase A: levels 1-2, per chunk ----------
            for cc in range(NCH):
                col0 = cc * CHUNK
                lrep = lpool.tile([128, CHUNK], F32, tag="lrep", bufs=2)
                nc.sync.dma_start(
                    out=lrep[:],
                    in_=lds[cc][:, :].unsqueeze(0)
                        .broadcast_to([32, V, CHUNK]))

                e2 = e2pool.tile([128, CHUNK], F32R, tag="e2", bufs=2)
                nc.scalar.activation(e2[:], lrep[:], EXP, scale=sct2[:, 0:1])

                for h in range(CHUNK // HALF):
                    hc = h * HALF
                    e1s = []
                    for t in range(NT1):
                        e1 = e1pool.tile([128, HALF], F32R, tag="e1", bufs=5)
                        nc.scalar.activation(e1[:], lrep[:, hc:hc + HALF],
                                             EXP, scale=sct1[:, t:t + 1])
                        e1s.append(e1)
                    ps1 = ppool.tile([128, HALF], F32, tag="ps")
                    for s in range(HALF // MMN):
                        scol = s * MMN
                        for t in range(NT1):
                            nc.tensor.matmul(
                                ps1[:, scol:scol + MMN],
                                g1t[:, 128 * t:128 * (t + 1)],
                                e1s[t][:, scol:scol + MMN],
                                start=(t == 0), stop=(t == NT1 - 1))
                    # X2 = E2x * psum1 (in place into e2)
                    nc.vector.tensor_mul(
                        e2[:, hc:hc + HALF], e2[:, hc:hc + HALF],
                        ps1[:].bitcast(F32R))
                    ps2 = ppool.tile([32, HALF], F32, tag="ps")
                    for s in range(HALF // MMN):
                        scol = s * MMN
                        nc.tensor.matmul(
                            ps2[:, scol:scol + MMN], g2t[:],
                            e2[:, hc + scol:hc + scol + MMN],
                            start=True, stop=True)
                    # drain f2 (chunk-packed rows 32c+m2)
                    nc.vector.tensor_copy(
                        f2all[32 * cc:32 * cc + 32, hc:hc + HALF], ps2[:])

            # ---------- phase B: levels 3-4, chunk-packed ----------
            # phase-B exp inputs (depend only on ld)
            l3x = spool.tile([128, CHUNK], F32R, tag="l3x", bufs=1)
            for cc in range(NCH):
                col0 = cc * CHUNK
                nc.sync.dma_start(
                    out=l3x[32 * cc:32 * (cc + 1), :].bitcast(F32),
                    in_=lds[cc][:, :].unsqueeze(0)
                        .broadcast_to([8, V, CHUNK]))
            nc.scalar.activation(l3x[:], l3x[:].bitcast(F32), EXP,
                                 scale=sct3[:, 0:1])

            l4x = spool.tile([32, CHUNK], F32R, tag="l4x", bufs=1)
            for cc in range(NCH):
                col0 = cc * CHUNK
                nc.sync.dma_start(out=l4x[5 * cc:5 * cc + 4, :].bitcast(F32),
                                  in_=lds[cc][:, :])
                nc.sync.dma_start(
                    out=l4x[5 * cc + 4:5 * cc + 5, :].bitcast(F32),
                    in_=lds[cc][0:1, :])
            # rows 20..31 only need finite values (scale=0): reuse ld chunk 0
            nc.sync.dma_start(
                out=l4x[20:32, :].bitcast(F32),
                in_=lds[0][:, :].unsqueeze(0).broadcast_to([3, V, CHUNK]))
            nc.scalar.activation(l4x[:], l4x[:].bitcast(F32), EXP,
                                 scale=sct4[:, 0:1])


            for h in range(CHUNK // HALF):
                hc = h * HALF
                # X3 = E3x * f2all (in place, per half; waits on the last
                # chunk's h-drain only)
                nc.vector.tensor_mul(l3x[:, hc:hc + HALF], l3x[:, hc:hc + HALF],
                                     f2all[:, hc:hc + HALF].bitcast(F32R))
                ps3 = ppool.tile([32, HALF], F32, tag="ps")
                for s in range(HALF // MMN):
                    scol = s * MMN
                    nc.tensor.matmul(ps3[:, scol:scol + MMN], g3t[:],
                                     l3x[:, hc + scol:hc + scol + MMN],
                                     start=True, stop=True)
                # X4 = E4x * psum3 (in place into l4x half)
                nc.vector.tensor_mul(l4x[:, hc:hc + HALF],
                                     l4x[:, hc:hc + HALF],
                                     ps3[:].bitcast(F32R))
                ps4 = ppool.tile([NCH, HALF], F32, tag="ps")
                for s in range(HALF // MMN):
                    scol = s * MMN
                    nc.tensor.matmul(ps4[:, scol:scol + MMN], g4t[:],
                                     l4x[:, hc + scol:hc + scol + MMN],
                                     start=True, stop=True)
                outsb = spool.tile([NCH, HALF], F32, tag="outsb", bufs=2)
                nc.vector.tensor_copy(outsb[:], ps4[:])
                nc.sync.dma_start(
                    out=y[:].rearrange("(c i) -> c i", i=CHUNK)[:, hc:hc + HALF],
                    in_=outsb[:])

    nc.compile()
    return nc


def kernel(x, lam0, lam1, pow1, lam2, pow2, lam3, pow3, lam4, pow4):
    x = np.asarray(x, np.float32)
    consts = build_constants(
        np.asarray(lam0, np.float32), np.asarray(lam1, np.float32),
        np.asarray(pow1, np.float32), np.asarray(lam2, np.float32),
        np.asarray(pow2, np.float32), np.asarray(lam3, np.float32),
        np.asarray(pow3, np.float32), np.asarray(lam4, np.float32),
        np.asarray(pow4, np.float32))

    nc = build_bass()

    in_maps = []
    for k in range(M_CORES):
        shard = x[k * BS:(k + 1) * BS, :]
        m = {"xt": np.ascontiguousarray(shard.T)}
        m.update(consts)
        in_maps.append(m)

    from concourse.bass_utils import run_bass_kernel_spmd
    res = run_bass_kernel_spmd(nc, in_maps, list(range(M_CORES)))
    out = np.concatenate([res.results[k]["y"] for k in range(M_CORES)])
    return out[:, None].astype(np.float32)


if __name__ == "__main__":
    import reference
    inputs = {k: np.asarray(v) for k, v in reference.setup_inputs().items()}
    got = kernel(**inputs)
    exp = np.asarray(reference.reference(**inputs))
    err = np.abs(got - exp).max() / (np.abs(exp).max() + 1e-30)
    print("shape", got.shape, "relerr", err)



# revision 17
# speedup vs baseline: 1.2420x; 1.2420x over previous
"""Trainium2 Bass kernel for nn_NestedFormula (basis-function formulation).

Tree: DEPTH=4, V=4. Level sizes n4=1, n3=5, n2=25, n1=125, n0=125.
  f1[n] = sum_v lam1[n,v] * x_v^pow1[n,v] + lam0[n]
  fd[n] = sum_v lamd[n,v] * x_v^powd[n,v] * f_{d-1}[5n+v] + f_{d-1}[5n+4]
  out   = f4[0]                          (per batch element)

Key idea: on x in [0.5, 1.5], x^p == sum_k c_k(p) * x^{a_k} for 8 shared
basis exponents a_k (a_0 = 0), fit error ~1e-9, |c| <= ~2.  Level 1 (500
exps in the reference) collapses into one K=32 matmul per chunk against a
feature tile P, gathered straight into level-2 order.  Levels 2..4 keep
scalar-engine exps (E2/E34, built from replicated ln x) because PSUM
results can only be combined with SBUF operands on the vector engine
(1-PSUM-operand rule), then reduce via PE matmuls.

Layout (per core, BS=16384 as 4 chunks of 4096 on partition bands):
  P   [128, 4096] rows 32c+4k+v = x_v^{a_k} of chunk c      (ln+exp, scalar)
  e2s [128, 4096] per chunk: rows 4n+v = x_v^pow2[n,v], rows 100+: 1
  e34s[128, 4096] rows 32c+m2 = E3 (m2<20), rows 32c+25+u = E4, else 1
  per 1024-col block bp (4 blocks):
    ps1_c = W1g.T @ P[32c]   f1 gathered, 128 rows           (2 mms) x4
    x2_c  = e2s_c * ps1_c                                    (DVE)
    f2g   = sum_c G2l_c.T @ x2_c   f2 gathered, band 32c     (8 mms)
    x3    = e34s * f2g                                       (DVE)
    ps3   = G3l.T @ x3   f3 at rows 32c+25+u                 (2 mms)
    x4    = e34s * ps3                                       (DVE)
    psy   = G4l.T @ x4   row c = y of chunk c                (2 mms)
    ysb   = copy(psy)    (scalar)  -> DMA out
"""
import numpy as np

import concourse.bacc as bacc
import concourse.mybir as mybir
from concourse.tile import TileContext

DEPTH = 4
V = 4
B = 131072
M_CORES = 8
BS = B // M_CORES          # 16384 per core
CHUNK = 4096
NCH = BS // CHUNK          # 4
MMN = 512                  # matmul free dim (one PSUM bank)
BP = 1024                  # elementwise block width
NBP = CHUNK // BP          # 4
HW_ = 2048                 # activation half width

F32 = mybir.dt.float32
F32R = mybir.dt.float32r

KB = 8                     # basis size (a_0 = 0)


def _sigma1(m):
    # psum1 row m -> level-1 node index (f1 gather)
    if m < 100:
        return 5 * (m // 4) + (m % 4)
    return 5 * (m - 100) + 4


def _tau2(m):
    # f2g row m -> level-2 node index (f2 gather)
    if m < 20:
        return 5 * (m // 4) + (m % 4)
    return 5 * (m - 20) + 4


def _fit_basis(pows_all):
    pmin, pmax = pows_all.min() - 0.1, pows_all.max() + 0.1
    k = np.arange(KB - 1)
    anz = 0.5 * (pmin + pmax) + 0.5 * (pmax - pmin) * np.cos(
        np.pi * (2 * k + 1) / (2 * (KB - 1)))
    a = np.concatenate([[0.0], np.sort(anz)])
    ngrid = 257
    s = np.cos(np.pi * (2 * np.arange(ngrid) + 1) / (2 * ngrid))
    xg = 1.0 + 0.5 * s
    A = xg[:, None] ** a[None, :]

    def coeffs(p):
        T = xg[:, None] ** p.ravel()[None, :]
        C, *_ = np.linalg.lstsq(A, T, rcond=None)
        return C.T.reshape(p.shape + (KB,)).astype(np.float64)

    return a, coeffs


def build_constants(lam0, lam1, pow1, lam2, pow2, lam3, pow3, lam4, pow4):
    lam0, lam1, lam2, lam3, lam4 = [
        np.asarray(z, np.float64) for z in (lam0, lam1, lam2, lam3, lam4)]
    pow2, pow3, pow4 = [np.asarray(p, np.float64) for p in (pow2, pow3, pow4)]
    a, coeffs = _fit_basis(np.asarray(pow1).ravel())
    C1 = coeffs(np.asarray(pow1))      # (125, 4, KB)

    c = {}
    # avec: per-partition exponent for the P-build Exp (rows 32c+4k+v -> a_k)
    avec = np.zeros((128, 1), np.float32)
    for cc in range(NCH):
        for k in range(KB):
            for v in range(V):
                avec[32 * cc + 4 * k + v, 0] = a[k]
    c["avec"] = avec

    # q2vec: E2 exp scales, rows 4n+v = pow2[n,v], rows 100+ -> 0 (E2=1)
    q2vec = np.zeros((128, 1), np.float32)
    for n in range(25):
        for v in range(V):
            q2vec[4 * n + v, 0] = pow2[n, v]
    c["q2vec"] = q2vec

    # q34vec: rows 32c+m2 -> pow3 (m2<20), rows 32c+25+u -> pow4, else 0
    q34vec = np.zeros((128, 1), np.float32)
    for cc in range(NCH):
        r0 = 32 * cc
        for m2 in range(20):
            q34vec[r0 + m2, 0] = pow3[m2 // 4, m2 % 4]
        for u in range(4):
            q34vec[r0 + 25 + u, 0] = pow4[0, u]
    c["q34vec"] = q34vec

    # W1g [32, 128]: col m = gathered f1 (sigma1); rows 4k+v = feature (k,v)
    w1g = np.zeros((32, 128), np.float64)
    for m in range(125):
        n = _sigma1(m)
        for v in range(V):
            for k in range(KB):
                w1g[4 * k + v, m] += lam1[n, v] * C1[n, v, k]
        w1g[0, m] += lam0[n]           # feature (k=0,v=0) == 1
    c["w1g"] = w1g.astype(np.float32)
    # chunk-3 variant: full-K lhsT, weights at rows 96..127 (PE tile base
    # 96 is not encodable)
    w1g3 = np.zeros((128, 128), np.float64)
    w1g3[96:128, :] = w1g
    c["w1g3"] = w1g3.astype(np.float32)

    # G2l per chunk: [128, 4*128], chunk c cols 128c+(32c+j) = f2 gather
    g2l = np.zeros((128, 32), np.float64)
    for m2 in range(25):
        n2t = _tau2(m2)
        for v in range(V):
            g2l[4 * n2t + v, m2] = lam2[n2t, v]
        g2l[100 + n2t, m2] = 1.0       # + f1[5*n2t+4]
    g2l4 = np.zeros((128, 4 * 128), np.float64)
    for cc in range(NCH):
        g2l4[:, 128 * cc + 32 * cc: 128 * cc + 32 * cc + 32] = g2l
    c["g2l4"] = g2l4.astype(np.float32)

    # G3l [128, 128] block-diag: col 32c+25+n3 = f3[n3] of chunk c
    g3l = np.zeros((128, 128), np.float64)
    for cc in range(NCH):
        r0 = 32 * cc
        for n3 in range(5):
            for v in range(V):
                g3l[r0 + 4 * n3 + v, r0 + 25 + n3] = lam3[n3, v]
            g3l[r0 + 20 + n3, r0 + 25 + n3] = 1.0
    c["g3l"] = g3l.astype(np.float32)

    # G4l [128, 4]: col c = y of chunk c
    g4l = np.zeros((128, 4), np.float64)
    for cc in range(NCH):
        r0 = 32 * cc
        for u in range(4):
            g4l[r0 + 25 + u, cc] = lam4[0, u]
        g4l[r0 + 29, cc] = 1.0
    c["g4l"] = g4l.astype(np.float32)
    return c


def build_bass():
    nc = bacc.Bacc()
    xt = nc.dram_tensor("xt", (V, BS), F32, kind="ExternalInput")
    avec = nc.dram_tensor("avec", (128, 1), F32, kind="ExternalInput")
    q2vec = nc.dram_tensor("q2vec", (128, 1), F32, kind="ExternalInput")
    q34vec = nc.dram_tensor("q34vec", (128, 1), F32, kind="ExternalInput")
    w1g = nc.dram_tensor("w1g", (32, 128), F32R, kind="ExternalInput")
    w1g3 = nc.dram_tensor("w1g3", (128, 128), F32R, kind="ExternalInput")
    g2l4 = nc.dram_tensor("g2l4", (128, 4 * 128), F32R, kind="ExternalInput")
    g3l = nc.dram_tensor("g3l", (128, 128), F32R, kind="ExternalInput")
    g4l = nc.dram_tensor("g4l", (128, 4), F32R, kind="ExternalInput")
    y = nc.dram_tensor("y", (BS,), F32, kind="ExternalOutput")

    EXP = mybir.ActivationFunctionType.Exp
    LN = mybir.ActivationFunctionType.Ln
    COPY = mybir.ActivationFunctionType.Copy

    with TileContext(nc) as tc:
        with tc.tile_pool(name="const", bufs=1) as cpool, \
             tc.tile_pool(name="sb", bufs=1) as spool, \
             tc.tile_pool(name="dram", bufs=1, space="DRAM") as dpool, \
             tc.tile_pool(name="psA", bufs=1, space="PSUM") as ppA, \
             tc.tile_pool(name="psB", bufs=1, space="PSUM") as ppB:

            # ---------- constants ----------
            av = cpool.tile([128, 1], F32)
            nc.sync.dma_start(out=av[:], in_=avec[:, :])
            q2v = cpool.tile([128, 1], F32, tag="q2v")
            nc.sync.dma_start(out=q2v[:], in_=q2vec[:, :])
            q34v = cpool.tile([128, 1], F32, tag="q34v")
            nc.sync.dma_start(out=q34v[:], in_=q34vec[:, :])

            def load_c(dt, shape, tag):
                r = cpool.tile(list(shape), F32R, tag=tag)
                nc.sync.dma_start(out=r[:], in_=dt[:, :])
                return r

            # w1g replicated to bands 0/32/64 (lhsT base == rhs base rule)
            w1gt = cpool.tile([128, 128], F32R, tag="w1g")
            nc.sync.dma_start(
                out=w1gt[:96, :],
                in_=w1g[:, :].unsqueeze(0).broadcast_to([3, 32, 128]))
            w1gt3 = load_c(w1g3, (128, 128), "w1g3")
            g2lt = load_c(g2l4, (128, 4 * 128), "g2l4")
            g3lt = load_c(g3l, (128, 128), "g3l")
            g4lt = load_c(g4l, (128, 4), "g4l")

            # ---------- P features + replicated-t exp tiles ----------
            xr = cpool.tile([128, CHUNK], F32, tag="xr")     # x -> ln x
            P = cpool.tile([128, CHUNK], F32R, tag="P")
            t2 = [cpool.tile([128, CHUNK], F32, tag=f"t2_{cc}",
                             name=f"t2_{cc}")
                  for cc in range(NCH)]                      # -> E2 per chunk
            t34 = cpool.tile([128, CHUNK], F32, tag="t34")   # -> E34

            for h in range(2):
                h0 = h * HW_
                for cc in range(NCH):
                    xv = xt[:, cc * CHUNK + h0: cc * CHUNK + h0 + HW_] \
                        .unsqueeze(0).broadcast_to([KB, V, HW_])
                    nc.sync.dma_start(
                        out=xr[32 * cc:32 * cc + 32, h0:h0 + HW_], in_=xv)
                nc.scalar.activation(xr[:, h0:h0 + HW_], xr[:, h0:h0 + HW_],
                                     LN)
                nc.scalar.activation(P[:, h0:h0 + HW_], xr[:, h0:h0 + HW_],
                                     EXP, scale=av[:, 0:1])
                for cc in range(NCH):
                    # bounce t (ln x) via DRAM: SBUF-source DMAs cannot
                    # broadcast along partitions
                    td = dpool.tile([4, HW_], F32, tag=f"td{cc}_{h}",
                                    name=f"td{cc}_{h}")
                    nc.sync.dma_start(out=td[:, :],
                                      in_=xr[32 * cc:32 * cc + 4,
                                             h0:h0 + HW_])
                    # t2_c rows 4n+v = t_v of chunk c (32x replication)
                    nc.gpsimd.dma_start(
                        out=t2[cc][:, h0:h0 + HW_],
                        in_=td[:, :].unsqueeze(0).broadcast_to([32, 4, HW_]))
                    nc.scalar.activation(
                        t2[cc][:, h0:h0 + HW_].bitcast(F32R),
                        t2[cc][:, h0:h0 + HW_], EXP, scale=q2v[:, 0:1])
                    # t34 rows 32c+m2: m2<20 <- t_{m2%4}; 20..31 filler;
                    # rows 25..28 then overwritten with t_u
                    nc.gpsimd.dma_start(
                        out=t34[32 * cc:32 * cc + 20, h0:h0 + HW_],
                        in_=td[:, :].unsqueeze(0).broadcast_to([5, 4, HW_]))
                    nc.gpsimd.dma_start(
                        out=t34[32 * cc + 20:32 * cc + 32, h0:h0 + HW_],
                        in_=td[:, :].unsqueeze(0).broadcast_to([3, 4, HW_]))
                    nc.gpsimd.dma_start(
                        out=t34[32 * cc + 25:32 * cc + 29, h0:h0 + HW_],
                        in_=td[:, :])
                nc.scalar.activation(t34[:, h0:h0 + HW_].bitcast(F32R),
                                     t34[:, h0:h0 + HW_], EXP,
                                     scale=q34v[:, 0:1])

            # ---------- main loop over 1024-col blocks ----------
            for bp in range(NBP):
                b0 = bp * BP
                x2s = []
                for cc in range(NCH):
                    r0 = 32 * cc
                    ps1 = ppA.tile([128, BP], F32, tag="ps1", bufs=2)
                    for s in range(BP // MMN):
                        sl = slice(b0 + s * MMN, b0 + (s + 1) * MMN)
                        if cc < 3:
                            nc.tensor.matmul(ps1[:, s * MMN:(s + 1) * MMN],
                                             w1gt[r0:r0 + 32, :],
                                             P[r0:r0 + 32, sl],
                                             start=True, stop=True)
                        else:
                            nc.tensor.matmul(ps1[:, s * MMN:(s + 1) * MMN],
                                             w1gt3[:], P[:, sl],
                                             start=True, stop=True)
                    x2 = spool.tile([128, BP], F32R, tag=f"x2_{cc}", bufs=2)
                    nc.vector.tensor_mul(x2[:],
                                         t2[cc][:, b0:b0 + BP].bitcast(F32R),
                                         ps1[:].bitcast(F32R))
                    x2s.append(x2)
                f2g = ppA.tile([128, BP], F32, tag="f2g", bufs=1)
                for s in range(BP // MMN):
                    for cc in range(NCH):
                        nc.tensor.matmul(
                            f2g[:, s * MMN:(s + 1) * MMN],
                            g2lt[:, 128 * cc:128 * cc + 128],
                            x2s[cc][:, s * MMN:(s + 1) * MMN],
                            start=(cc == 0), stop=(cc == 3))
                x3 = spool.tile([128, BP], F32R, tag="x3", bufs=2)
                nc.vector.tensor_mul(x3[:], t34[:, b0:b0 + BP].bitcast(F32R),
                                     f2g[:].bitcast(F32R))
                ysb = spool.tile([4, BP], F32, tag="ysb", bufs=2)
                for s in range(BP // MMN):
                    ss = slice(s * MMN, (s + 1) * MMN)
                    ps3 = ppB.tile([128, MMN], F32, tag="ps3", bufs=1)
                    nc.tensor.matmul(ps3[:], g3lt[:], x3[:, ss],
                                     start=True, stop=True)
                    x4 = spool.tile([128, MMN], F32R, tag="x4", bufs=2)
                    nc.vector.tensor_mul(
                        x4[:], t34[:, b0 + s * MMN:b0 + (s + 1) * MMN]
                        .bitcast(F32R), ps3[:].bitcast(F32R))
                    psy = ppB.tile([4, MMN], F32, tag="psy", bufs=1)
                    nc.tensor.matmul(psy[:], g4lt[:], x4[:],
                                     start=True, stop=True)
                    nc.scalar.activation(ysb[:, ss], psy[:], COPY)
                nc.sync.dma_start(
                    out=y[:].rearrange("(c i) -> c i", i=CHUNK)[:, b0:b0 + BP],
                    in_=ysb[:])

    nc.compile()
    return nc


def kernel(x, lam0, lam1, pow1, lam2, pow2, lam3, pow3, lam4, pow4):
    x = np.asarray(x, np.float32)
    consts = build_constants(lam0, lam1, pow1, lam2, pow2,
                             lam3, pow3, lam4, pow4)
    nc = build_bass()

    in_maps = []
    for k in range(M_CORES):
        shard = x[k * BS:(k + 1) * BS, :]
        m = {"xt": np.ascontiguousarray(shard.T)}
        m.update(consts)
        in_maps.append(m)

    from concourse.bass_utils import run_bass_kernel_spmd
    res = run_bass_kernel_spmd(nc, in_maps, list(range(M_CORES)))
    out = np.concatenate([res.results[k]["y"] for k in range(M_CORES)])
    return out[:, None].astype(np.float32)


if __name__ == "__main__":
    import reference
    inputs = {k: np.asarray(v) for k, v in reference.setup_inputs().items()}
    got = kernel(**inputs)
    exp = np.asarray(reference.reference(**inputs))
    err = np.abs(got - exp).max() / (np.abs(exp).max() + 1e-30)
    print("shape", got.shape, "relerr", err)
